# revision 10
# baseline (speedup 1.0000x reference)
"""Trainium2 Bass kernel for nn_CRF: fc projection + Viterbi decode.

Algorithm (device):
  - fc: emit = features @ W.T + b, data-parallel over L across 8 cores.
  - Viterbi forward scan parallelized via batched speculative chunking:
    each core's 4096 rows split into 128 subchunks of 32 steps, one per
    SBUF partition; every vector instruction advances 128 independent
    scans one step.  Max-plus coalescence makes a short warmup converge
    to the true state up to a uniform additive constant, which argmax
    decisions are invariant to (within-binade fp argument) -- two passes:
      pass 1: zero-init warmup scan -> per-subchunk coalesced state
              shapes + per-subchunk level gains
      host:   prefix-sum of gains -> absolute level offsets
      pass 2: scan re-initialized at true magnitude -> exact backpointers
  - argmax per step computed exactly (first-tie like jnp.argmax) via
    key = (max - score) * 2^28 + j, reduce_min.
  - host: tiny glue (offset prefix over 1024 scalars), backtrack over
    device-produced backpointers, exact fp32 path-score chain (cumsum).

Scheduling note: every DMA-produced tile is "funneled" through one DVE
tensor_copy before compute consumes it, so PE/DVE instructions carry at
most one sync wait (the walrus LDW slot fits a single wait; multi-queue
DMA fanout otherwise produces several).
"""

import os
import numpy as np

import concourse.bass as bass
import concourse.mybir as mybir
from concourse import bacc
from concourse.tile import TileContext
from concourse.bass import AP
from concourse.bass_utils import run_bass_kernel_spmd

F32 = mybir.dt.float32

# ---------------------------------------------------------------- config

class CFG:
    L = 32768          # sequence length
    D = 1024           # feature dim
    C = 50             # tags (incl START=48, STOP=49)
    NC = 8             # cores
    NP = 128           # partitions used per core
    SUB = 32           # subchunk length  (NP*SUB = L/NC rows per core)
    PAD = 128          # emit rows computed before each core's chunk
    W1 = 16            # pass-1 warmup steps
    W2 = 16            # pass-2 warmup steps
    KTILE = 128        # matmul contraction tile

    @property
    def ROWS(self):
        return self.NP * self.SUB

    @property
    def KT(self):
        return self.D // self.KTILE

    @property
    def RT(self):
        return (self.PAD + self.ROWS) // 128


IMP = np.float32(-10000.0)
BIGK = float(2 ** 28)


def _fv0(c):
    fv = np.full(c.C, IMP, dtype=np.float32)
    fv[c.C - 2] = np.float32(0.0)        # START
    return fv


# ---------------------------------------------------------------- builders

def _emit48_src(c, emit_d, steps):
    """AP reading emit rows [SUB*p - W + s] for p in [0,NP), s in [0,steps)."""
    off = (c.PAD - (steps - c.SUB)) * c.C
    return AP(emit_d[:, :].tensor, off,
              [[c.SUB * c.C, c.NP], [c.C, steps], [1, c.C]])


def _funnel(nc, pool, shape, dtype, src_ap, tag):
    """DMA src -> raw tile, then one DVE copy -> shadow tile.

    Compute reads the shadow, so its dependency is a single DVE-sem tick
    instead of several DMA-queue sems.
    """
    raw = pool.tile(shape, dtype, tag=tag + "_raw")
    nc.sync.dma_start(out=raw[:, :], in_=src_ap)
    shadow = pool.tile(shape, dtype, tag=tag)
    nc.vector.tensor_copy(out=shadow[:, :], in_=raw[:, :])
    return shadow


def _p0fix_ops(nc, pool, c, FV, fm_sb):
    """FV[0:1,:] = FV[0:1,:]*mask + fix  (uniform SPMD core-0 special-case).

    fm_sb is a funneled [1, C+1] tile: [:C] = fix row, [C:] = mask scalar.
    """
    tmp = pool.tile([1, c.C], F32, tag="tmpfix")
    nc.vector.scalar_tensor_tensor(
        out=tmp[:, :], in0=FV[0:1, :], scalar=fm_sb[0:1, c.C:c.C + 1],
        in1=fm_sb[0:1, :c.C],
        op0=mybir.AluOpType.mult, op1=mybir.AluOpType.add)
    nc.vector.tensor_copy(out=FV[0:1, :], in_=tmp[:, :])


def build_A(c):
    """fc matmul + pass-1 scan (shapes + gains)."""
    nc = bacc.Bacc("TRN2", debug=False)
    NR = c.PAD + c.ROWS
    fT_d = nc.dram_tensor("featsT", [c.D, NR], F32, kind="ExternalInput")
    wT_d = nc.dram_tensor("wT", [c.D, c.C], F32, kind="ExternalInput")
    b_d = nc.dram_tensor("bias", [1, c.C], F32, kind="ExternalInput")
    T_d = nc.dram_tensor("tmat", [1, c.C * c.C], F32, kind="ExternalInput")
    init_d = nc.dram_tensor("init1", [c.NP, c.C], F32, kind="ExternalInput")
    fm_d = nc.dram_tensor("p0fm", [1, c.C + 1], F32, kind="ExternalInput")
    emit_d = nc.dram_tensor("emit", [NR, c.C], F32, kind="ExternalOutput")
    smid_d = nc.dram_tensor("smid", [c.NP, c.C], F32, kind="ExternalOutput")
    sat_d = nc.dram_tensor("sat", [c.NP, c.C], F32, kind="ExternalOutput")
    send_d = nc.dram_tensor("send", [c.NP, c.C], F32, kind="ExternalOutput")

    STEPS = c.SUB + c.W1
    with TileContext(nc) as tc:
        with tc.tile_pool(name="sbuf", bufs=1) as pool, \
             tc.tile_pool(name="fcin", bufs=8) as fcin, \
             tc.tile_pool(name="fcsh", bufs=3) as fcsh, \
             tc.tile_pool(name="psum", bufs=4, space="PSUM") as pp:
            # ---- fc: emit = featsT.T @ wT + b
            wt_sb = _funnel(nc, pool, [c.KTILE, c.KT * c.C], F32,
                            wT_d[:, :].rearrange("(k p) c -> p k c", p=c.KTILE),
                            "wt")
            bias_rep = _funnel(nc, pool, [128, c.C], F32,
                               b_d[:, :].to_broadcast((128, c.C)), "bias")
            emit_lin = pool.tile([128, c.RT * c.C], F32)
            wtv = wt_sb[:, :].rearrange("p (k c) -> p k c", c=c.C)
            for rt in range(c.RT):
                raw = fcin.tile([c.KTILE, c.KT * 128], F32, tag="ftraw")
                nc.sync.dma_start(
                    out=raw[:, :].rearrange("p (k r) -> p k r", r=128),
                    in_=fT_d[:, rt * 128:(rt + 1) * 128]
                        .rearrange("(k p) r -> p k r", p=c.KTILE))
                ft2 = fcsh.tile([c.KTILE, c.KT * 128], F32, tag="ft2")
                nc.vector.tensor_copy(out=ft2[:, :], in_=raw[:, :])
                ps = pp.tile([128, c.C], F32, tag="ps")
                ftv = ft2[:, :].rearrange("p (k r) -> p k r", r=128)
                for k in range(c.KT):
                    nc.tensor.matmul(ps[:, :], ftv[:, k, :], wtv[:, k, :],
                                     start=(k == 0), stop=(k == c.KT - 1))
                nc.vector.tensor_add(out=emit_lin[:, rt * c.C:(rt + 1) * c.C],
                                     in0=ps[:, :], in1=bias_rep[:, :])
            nc.sync.dma_start(
                out=emit_d[:, :].rearrange("(rt p) c -> p rt c", p=128),
                in_=emit_lin[:, :].rearrange("p (rt c) -> p rt c", c=c.C))

            # ---- pass-1 scan
            T_rep = _funnel(nc, pool, [c.NP, c.C * c.C], F32,
                            T_d[:, :].to_broadcast((c.NP, c.C * c.C)), "trep")
            emit48 = pool.tile([c.NP, STEPS * c.C], F32)
            nc.sync.dma_start(
                out=emit48[:, :].rearrange("p (s c) -> p s c", c=c.C),
                in_=_emit48_src(c, emit_d, STEPS))
            e48sh = pool.tile([c.NP, STEPS * c.C], F32)
            nc.vector.tensor_copy(out=e48sh[:, :], in_=emit48[:, :])
            FV = _funnel(nc, pool, [c.NP, c.C], F32, init_d[:, :], "fv")
            fm_sb = _funnel(nc, pool, [1, c.C + 1], F32, fm_d[:, :], "fm")
            S3 = pool.tile([c.NP, c.C * c.C], F32)
            m = pool.tile([c.NP, c.C], F32)
            T3 = T_rep[:, :].rearrange("p (i j) -> p i j", j=c.C)
            S3v = S3[:, :].rearrange("p (i j) -> p i j", j=c.C)
            e48 = e48sh[:, :].rearrange("p (s c) -> p s c", c=c.C)
            for s in range(STEPS):
                if s == c.W1:
                    _p0fix_ops(nc, pool, c, FV, fm_sb)
                    nc.sync.dma_start(out=smid_d[:, :], in_=FV[:, :])
                if s == c.W1 + c.SUB - c.W2:
                    nc.sync.dma_start(out=sat_d[:, :], in_=FV[:, :])
                nc.vector.tensor_add(
                    out=S3v, in0=T3,
                    in1=FV[:, None, :].broadcast_to([c.NP, c.C, c.C]))
                nc.vector.tensor_reduce(out=m[:, :], in_=S3v,
                                        axis=mybir.AxisListType.X,
                                        op=mybir.AluOpType.max)
                nc.vector.tensor_add(out=FV[:, :], in0=m[:, :], in1=e48[:, s, :])
            nc.sync.dma_start(out=send_d[:, :], in_=FV[:, :])
    return nc


def build_B(c):
    """pass-2 scan: exact backpointers."""
    nc = bacc.Bacc("TRN2", debug=False)
    NR = c.PAD + c.ROWS
    emit_d = nc.dram_tensor("emit", [NR, c.C], F32, kind="ExternalInput")
    T_d = nc.dram_tensor("tmat", [1, c.C * c.C], F32, kind="ExternalInput")
    iota_d = nc.dram_tensor("iota", [1, c.C * c.C], F32, kind="ExternalInput")
    init_d = nc.dram_tensor("init2", [c.NP, c.C], F32, kind="ExternalInput")
    fm_d = nc.dram_tensor("p0fm", [1, c.C + 1], F32, kind="ExternalInput")
    bp_d = nc.dram_tensor("bp", [c.ROWS, c.C], F32, kind="ExternalOutput")
    fvout_d = nc.dram_tensor("fvout", [c.NP, c.C], F32, kind="ExternalOutput")

    STEPS = c.SUB + c.W2
    with TileContext(nc) as tc:
        with tc.tile_pool(name="sbuf", bufs=1) as pool:
            T_rep = _funnel(nc, pool, [c.NP, c.C * c.C], F32,
                            T_d[:, :].to_broadcast((c.NP, c.C * c.C)), "trep")
            iota_rep = _funnel(nc, pool, [c.NP, c.C * c.C], F32,
                               iota_d[:, :].to_broadcast((c.NP, c.C * c.C)),
                               "iota")
            emit48 = pool.tile([c.NP, STEPS * c.C], F32)
            nc.sync.dma_start(
                out=emit48[:, :].rearrange("p (s c) -> p s c", c=c.C),
                in_=_emit48_src(c, emit_d, STEPS))
            e48sh = pool.tile([c.NP, STEPS * c.C], F32)
            nc.vector.tensor_copy(out=e48sh[:, :], in_=emit48[:, :])
            FV = _funnel(nc, pool, [c.NP, c.C], F32, init_d[:, :], "fv")
            fm_sb = _funnel(nc, pool, [1, c.C + 1], F32, fm_d[:, :], "fm")
            S3 = pool.tile([c.NP, c.C * c.C], F32)
            D3 = pool.tile([c.NP, c.C * c.C], F32)
            K3 = pool.tile([c.NP, c.C * c.C], F32)
            m = pool.tile([c.NP, c.C], F32)
            bp_buf = pool.tile([c.NP, c.SUB * c.C], F32)
            T3 = T_rep[:, :].rearrange("p (i j) -> p i j", j=c.C)
            I3 = iota_rep[:, :].rearrange("p (i j) -> p i j", j=c.C)
            S3v = S3[:, :].rearrange("p (i j) -> p i j", j=c.C)
            D3v = D3[:, :].rearrange("p (i j) -> p i j", j=c.C)
            K3v = K3[:, :].rearrange("p (i j) -> p i j", j=c.C)
            e48 = e48sh[:, :].rearrange("p (s c) -> p s c", c=c.C)
            bpv = bp_buf[:, :].rearrange("p (s c) -> p s c", c=c.C)
            for s in range(STEPS):
                if s == c.W2:
                    _p0fix_ops(nc, pool, c, FV, fm_sb)
                nc.vector.tensor_add(
                    out=S3v, in0=T3,
                    in1=FV[:, None, :].broadcast_to([c.NP, c.C, c.C]))
                nc.vector.tensor_reduce(out=m[:, :], in_=S3v,
                                        axis=mybir.AxisListType.X,
                                        op=mybir.AluOpType.max)
                if s >= c.W2:
                    nc.vector.tensor_tensor(
                        out=D3v, in0=m[:, :, None].broadcast_to([c.NP, c.C, c.C]),
                        in1=S3v, op=mybir.AluOpType.subtract)
                    nc.vector.scalar_tensor_tensor(
                        out=K3v, in0=D3v, scalar=BIGK, in1=I3,
                        op0=mybir.AluOpType.mult, op1=mybir.AluOpType.add)
                    nc.vector.tensor_reduce(out=bpv[:, s - c.W2, :], in_=K3v,
                                            axis=mybir.AxisListType.X,
                                            op=mybir.AluOpType.min)
                nc.vector.tensor_add(out=FV[:, :], in0=m[:, :], in1=e48[:, s, :])
            nc.sync.dma_start(
                out=bp_d[:, :].rearrange("(p s) c -> p s c", p=c.NP),
                in_=bp_buf[:, :].rearrange("p (s c) -> p s c", c=c.C))
            nc.sync.dma_start(out=fvout_d[:, :], in_=FV[:, :])
    return nc


# ---------------------------------------------------------------- host glue

def _host_inputs_A(c, features, W, b, T):
    NR = c.PAD + c.ROWS
    fT = np.ascontiguousarray(features.T)          # [D, L]
    Tflat = np.ascontiguousarray(T.reshape(1, -1)).astype(np.float32)
    wT = np.ascontiguousarray(W.T).astype(np.float32)
    bR = np.ascontiguousarray(b.reshape(1, -1)).astype(np.float32)
    fv0 = _fv0(c)
    in_maps = []
    for k in range(c.NC):
        lo = k * c.ROWS - c.PAD
        ftk = np.zeros((c.D, NR), dtype=np.float32)
        src_lo = max(lo, 0)
        ftk[:, src_lo - lo:] = fT[:, src_lo:lo + NR]
        core0 = (k == 0)
        fm = np.zeros((1, c.C + 1), dtype=np.float32)
        fm[0, :c.C] = fv0 if core0 else 0.0
        fm[0, c.C] = 0.0 if core0 else 1.0
        in_maps.append({
            "featsT": ftk,
            "wT": wT,
            "bias": bR,
            "tmat": Tflat,
            "init1": np.zeros((c.NP, c.C), dtype=np.float32),
            "p0fm": fm,
        })
    return in_maps


def _host_glue(c, resA):
    """pass-1 outputs -> pass-2 init states per core."""
    S = c.NC * c.NP                    # total subchunks
    smid = np.concatenate([resA[k]["smid"] for k in range(c.NC)])
    sat = np.concatenate([resA[k]["sat"] for k in range(c.NC)])
    send = np.concatenate([resA[k]["send"] for k in range(c.NC)])
    midmax = smid.max(axis=1)
    gains = send.max(axis=1) - midmax
    Mp = np.concatenate([[0.0], np.cumsum(gains.astype(np.float64))])[:S]
    shift = (Mp - midmax.astype(np.float64)).astype(np.float32)
    init2 = np.zeros((S, c.C), dtype=np.float32)
    init2[0] = _fv0(c)
    init2[1:] = sat[:-1] + shift[:-1, None]
    return init2


def _host_inputs_B(c, resA, init2, T):
    Tflat = np.ascontiguousarray(T.reshape(1, -1)).astype(np.float32)
    iota = np.tile(np.arange(c.C, dtype=np.float32), c.C).reshape(1, -1)
    fv0 = _fv0(c)
    in_maps = []
    for k in range(c.NC):
        core0 = (k == 0)
        fm = np.zeros((1, c.C + 1), dtype=np.float32)
        fm[0, :c.C] = fv0 if core0 else 0.0
        fm[0, c.C] = 0.0 if core0 else 1.0
        in_maps.append({
            "emit": resA[k]["emit"],
            "tmat": Tflat,
            "iota": iota,
            "init2": np.ascontiguousarray(init2[k * c.NP:(k + 1) * c.NP]),
            "p0fm": fm,
        })
    return in_maps


def _finalize(c, resB, emit_full, T):
    """backtrack + exact fp32 path score from device backpointers."""
    bp = np.concatenate([resB[k]["bp"] for k in range(c.NC)])  # [L, C] f32
    bp = np.rint(bp).astype(np.int32)
    fv_final = resB[c.NC - 1]["fvout"][c.NP - 1]
    terminal = fv_final + T[c.C - 1]
    best = int(np.argmax(terminal))
    L = c.L
    path = np.empty(L, dtype=np.int32)
    tag = best
    for t in range(L - 1, -1, -1):
        path[t] = tag
        tag = bp[t, tag]
    # exact fp32 sequential score chain (matches the reference's fv rounding)
    terms = np.empty(2 * L + 1, dtype=np.float32)
    prev = np.concatenate([[c.C - 2], path[:-1]])
    terms[0:2 * L:2] = T[path, prev]
    terms[1:2 * L:2] = emit_full[np.arange(L), path]
    terms[2 * L] = T[c.C - 1, path[-1]]
    score = np.cumsum(terms, dtype=np.float32)[-1]
    return np.float32(score), path


# ---------------------------------------------------------------- runners

_cache = {}
LAST_EXEC_NS = []      # exec_time_ns (or wall ns fallback) per launch


def _get_programs(c):
    key = (c.L, c.D, c.C, c.NC, c.NP, c.SUB, c.PAD, c.W1, c.W2)
    if key not in _cache:
        ncA, ncB = build_A(c), build_B(c)
        ncA.finalize()
        ncB.finalize()
        _cache[key] = (ncA, ncB)
    return _cache[key]


def _run_spmd(nc, in_maps, core_ids):
    import time as _time
    trace = bool(os.environ.get("CRF_TRACE"))
    t0 = _time.monotonic_ns()
    try:
        r = run_bass_kernel_spmd(nc, in_maps, core_ids=core_ids, trace=trace)
    except ModuleNotFoundError:
        r = run_bass_kernel_spmd(nc, in_maps, core_ids=core_ids, trace=False)
    LAST_EXEC_NS.append(r.exec_time_ns if r.exec_time_ns is not None
                        else _time.monotonic_ns() - t0)
    return r.results


def run_pipeline(c, features, W, b, transitions, runner=_run_spmd):
    ncA, ncB = _get_programs(c)
    core_ids = list(range(c.NC))
    in_A = _host_inputs_A(c, features, W, b, transitions)
    resA = runner(ncA, in_A, core_ids)
    init2 = _host_glue(c, resA)
    in_B = _host_inputs_B(c, resA, init2, transitions)
    resB = runner(ncB, in_B, core_ids)
    emit_full = np.concatenate(
        [resA[k]["emit"][c.PAD:] for k in range(c.NC)])   # [L, C]
    score, path = _finalize(c, resB, emit_full, transitions)
    return score, path


def kernel(features, W, b, transitions):
    c = CFG()
    features = np.asarray(features, dtype=np.float32)
    W = np.asarray(W, dtype=np.float32)
    b = np.asarray(b, dtype=np.float32)
    transitions = np.asarray(transitions, dtype=np.float32)
    del LAST_EXEC_NS[:]
    score, path = run_pipeline(c, features, W, b, transitions)
    return score, path, transitions
